# revision 9
# baseline (speedup 1.0000x reference)
"""GraphConv + BatchNorm + LeakyReLU fused layer on 8 Trainium2 NeuronCores.

Strategy (node/edge-partition sharding, v2):
  - Destination nodes are sharded across the 8 cores (6250 each). Within a
    core, dst nodes are assigned to 98 blocks of width 64 by balanced
    (LPT-style) packing on in-degree so every block has nearly the same edge
    count; a uniform slot count L = ceil(max_block_edges/128) is baked into
    the program.
  - Each core fetches the bf16 source row for each of its edges with a single
    indirect DMA gather stream (int32 indices, no halving), one 256B
    descriptor per edge, into G tiles of [128 edge-lanes, slots, 128 feat].
  - The per-block segment sum is a PE matmul: aggT += G_t^T @ S_t where
    S_t[lane, d] = (dst_lane == d) is a [128, 64] one-hot built batched per
    block on DVE with a broadcast iota compare (64-wide S halves DVE work
    vs 128-wide blocks).
  - x1^T = W_rel^T.T @ aggT + W_root^T.T @ x_own^T accumulates in PSUM;
    x2 = leaky_relu(x1 + b_rel) is an ACT copy (bias) + one DVE
    scalar_tensor_tensor (max(0.01*v, v)); x3^T = W_lin^T.T @ x2.
  - The x3 block copy (ACT) accumulates per-feature sums; a Square activation
    accumulates sums of squares. Global BN stats go through a DRAM bounce +
    AllReduce; the final affine+leaky and a PE transpose per block-pair
    produce bf16 outputs stored per 128 rows (upcast to f32 on the host).

kernel(**inputs) takes full-size numpy inputs, returns [50000, 128] float32.
"""
import sys

if "/opt/trn_rl_repo" not in sys.path:
    sys.path.insert(0, "/opt/trn_rl_repo")

import numpy as np
import ml_dtypes

import concourse.bass as bass
import concourse.mybir as mybir
import concourse.tile as tile
from concourse import bacc
from concourse import bass_utils
from concourse.masks import make_identity

F32 = mybir.dt.float32
BF16 = mybir.dt.bfloat16
I32 = mybir.dt.int32

N_NODES = 50000
N_CORES = 8
NPC = N_NODES // N_CORES          # 6250 nodes per core
BW = 64                           # dst-block width (S matrix width)
NBLK = (NPC + BW - 1) // BW       # 98 dst blocks per core
NPC_PAD = NBLK * BW               # 6272
LAST_BLK = NPC - BW * (NBLK - 1)  # 42 real nodes in the last block
ROWS_PAD = ((N_NODES + 127) // 128) * 128  # 50048
CHUNK = 8                         # dst blocks per gather
BN_EPS = 1e-5
NEG = 0.01


def _chunks(nblk, chunk):
    out = []
    b = 0
    while b < nblk:
        out.append((b, min(chunk, nblk - b)))
        b += chunk
    return out


def build_program(cfg):
    """Build the SPMD Bass program. cfg keys: n_cores, nblk, l_max, chunk,
    n_total, has_brel, has_blin."""
    ncores = cfg["n_cores"]
    nblk = cfg["nblk"]
    L = cfg["l_max"]
    chunk = cfg["chunk"]
    npc_pad = nblk * BW
    chunks = _chunks(nblk, chunk)
    nslots = nblk * L

    nc = bacc.Bacc("TRN2", target_bir_lowering=False, debug=False,
                   num_devices=ncores)

    xb_d = nc.dram_tensor("xb", [ROWS_PAD, 128], BF16, kind="ExternalInput")
    xot_d = nc.dram_tensor("x_ownT", [128, npc_pad], BF16,
                           kind="ExternalInput")
    ix_d = nc.dram_tensor("idx", [128, nslots], I32, kind="ExternalInput")
    dv_d = nc.dram_tensor("dvals", [128, nslots], BF16, kind="ExternalInput")
    io_d = nc.dram_tensor("iota", [128, 128], BF16, kind="ExternalInput")
    wr_d = nc.dram_tensor("WrT", [128, 128], BF16, kind="ExternalInput")
    wo_d = nc.dram_tensor("WoT", [128, 128], BF16, kind="ExternalInput")
    wl_d = nc.dram_tensor("WlT", [128, 128], BF16, kind="ExternalInput")
    br_d = nc.dram_tensor("brel", [128, 1], F32, kind="ExternalInput")
    bl_d = nc.dram_tensor("blin", [128, 1], F32, kind="ExternalInput")
    ga_d = nc.dram_tensor("gamma", [128, 1], F32, kind="ExternalInput")
    be_d = nc.dram_tensor("beta", [128, 1], F32, kind="ExternalInput")
    out_d = nc.dram_tensor("out", [npc_pad, 128], BF16, kind="ExternalOutput")

    inv_n = 1.0 / float(cfg["n_total"])

    nchunks = len(chunks)

    with tile.TileContext(nc) as tc:
        with (
            tc.tile_pool(name="consts", bufs=1) as consts,
            tc.tile_pool(name="gp", bufs=2) as gp,
            tc.tile_pool(name="idxp", bufs=2) as idxp,
            tc.tile_pool(name="sp", bufs=2) as sp,
            tc.tile_pool(name="ps", bufs=2, space="PSUM") as ps,
            tc.tile_pool(name="tp", bufs=2, space="PSUM") as tp,
            tc.tile_pool(name="misc", bufs=3) as misc,
            tc.tile_pool(name="big", bufs=1) as big,
            tc.tile_pool(name="dram", bufs=1, space="DRAM") as dram,
        ):
            # ---- constants / persistent tiles ----
            io_s = consts.tile([128, 128], BF16)
            wr_s = consts.tile([128, 128], BF16)
            wo_s = consts.tile([128, 128], BF16)
            wl_s = consts.tile([128, 128], BF16)
            br_s = consts.tile([128, 1], F32)
            bl_s = consts.tile([128, 1], F32)
            ga_s = consts.tile([128, 1], F32)
            be_s = consts.tile([128, 1], F32)
            ident = consts.tile([128, 128], F32)
            dv_s = consts.tile([128, nslots], BF16)
            xot_s = big.tile([128, npc_pad], BF16)
            x3_s = big.tile([128, npc_pad], F32)
            sums = big.tile([128, nchunks + 1], F32)
            sqs = big.tile([128, nchunks + 1], F32)

            # first chunk's index load + gather go first so the gather's
            # descriptor generation overlaps the constant loads
            b0_0, nb_0 = chunks[0]
            ix_t0 = idxp.tile([128, chunk * L], I32, tag="ix")
            nc.sync.dma_start(ix_t0[:, 0:nb_0 * L], ix_d[:, 0:nb_0 * L])
            G0 = gp.tile([128, chunk * L, 128], BF16, tag="G")
            nc.gpsimd.indirect_dma_start(
                out=G0[:, 0:nb_0 * L, :],
                out_offset=None,
                in_=xb_d[:],
                in_offset=bass.IndirectOffsetOnAxis(
                    ap=ix_t0[:, 0:nb_0 * L], axis=0),
            )

            nc.sync.dma_start(dv_s[:], dv_d[:])
            nc.scalar.dma_start(io_s[:], io_d[:])
            nc.scalar.dma_start(wr_s[:], wr_d[:])
            nc.scalar.dma_start(wo_s[:], wo_d[:])
            nc.scalar.dma_start(wl_s[:], wl_d[:])
            nc.scalar.dma_start(br_s[:], br_d[:])
            nc.scalar.dma_start(bl_s[:], bl_d[:])
            nc.scalar.dma_start(ga_s[:], ga_d[:])
            nc.scalar.dma_start(be_s[:], be_d[:])
            nc.scalar.dma_start(xot_s[:], xot_d[:])
            make_identity(nc, ident[:])

            io_ap = io_s[:, 0:BW]
            brel = None
            blin = None

            for ci, (b0, nb) in enumerate(chunks):
                ncols = nb * L          # gather slots in this chunk
                nd = nb * BW            # dst columns in this chunk
                s0 = b0 * L
                d0 = b0 * BW
                # real (non-pad) dst columns in this chunk
                ndr = nd - (BW - LAST_BLK) if b0 + nb == nblk else nd
                if ci == 0:
                    G = G0
                else:
                    ix_t = idxp.tile([128, chunk * L], I32, tag="ix")
                    nc.sync.dma_start(ix_t[:, 0:ncols],
                                      ix_d[:, s0:s0 + ncols])
                    G = gp.tile([128, chunk * L, 128], BF16, tag="G")
                    nc.gpsimd.indirect_dma_start(
                        out=G[:, 0:ncols, :],
                        out_offset=None,
                        in_=xb_d[:],
                        in_offset=bass.IndirectOffsetOnAxis(
                            ap=ix_t[:, 0:ncols], axis=0),
                    )

                # ---- S tiles for the whole chunk (one DVE inst) ----
                S = sp.tile([128, chunk * L, BW], BF16, tag="S")
                dvb = dv_s[:, s0:s0 + ncols]
                iota_bc = bass.AP(tensor=io_ap.tensor, offset=io_ap.offset,
                                  ap=[io_ap.ap[0], [0, ncols], io_ap.ap[1]])
                dv_bc = bass.AP(tensor=dvb.tensor, offset=dvb.offset,
                                ap=[dvb.ap[0], dvb.ap[1], [0, BW]])
                nc.vector.tensor_tensor(out=S[:, 0:ncols, :], in0=iota_bc,
                                        in1=dv_bc,
                                        op=mybir.AluOpType.is_equal)

                # ---- segment-sum matmuls into one [128, nd] PSUM tile ----
                agg_ps = ps.tile([128, chunk * BW], F32, tag="agg")
                for b in range(nb):
                    for t in range(L):
                        nc.tensor.matmul(
                            agg_ps[:, b * BW:(b + 1) * BW],
                            lhsT=G[:, b * L + t, :],
                            rhs=S[:, b * L + t, :],
                            start=(t == 0), stop=(t == L - 1))
                aggT = misc.tile([128, chunk * BW], BF16, tag="aggT")
                nc.scalar.copy(aggT[:, 0:nd], agg_ps[:, 0:nd])

                # ---- x1^T = W_rel^T.T @ aggT + W_root^T.T @ x_own^T ----
                x1_ps = ps.tile([128, chunk * BW], F32, tag="x1")
                nc.tensor.matmul(x1_ps[:, 0:nd], lhsT=wr_s[:],
                                 rhs=aggT[:, 0:nd], start=True, stop=False)
                nc.tensor.matmul(x1_ps[:, 0:nd], lhsT=wo_s[:],
                                 rhs=xot_s[:, d0:d0 + nd],
                                 start=False, stop=True)

                # x2 = leaky(x1 + b_rel): ACT copy (bias) + DVE one-op leaky
                x2_sb = misc.tile([128, chunk * BW], BF16, tag="x2")
                v_sb = misc.tile([128, chunk * BW], BF16, tag="v")
                brel = br_s[:] if cfg["has_brel"] else 0.0
                nc.scalar.activation(
                    v_sb[:, 0:nd], x1_ps[:, 0:nd],
                    mybir.ActivationFunctionType.Identity,
                    bias=brel, scale=1.0)
                nc.vector.scalar_tensor_tensor(
                    out=x2_sb[:, 0:nd], in0=v_sb[:, 0:nd], scalar=NEG,
                    in1=v_sb[:, 0:nd],
                    op0=mybir.AluOpType.mult, op1=mybir.AluOpType.max)

                # x3^T = W_lin^T.T @ x2
                x3_ps = ps.tile([128, chunk * BW], F32, tag="x3")
                nc.tensor.matmul(x3_ps[:, 0:nd], lhsT=wl_s[:],
                                 rhs=x2_sb[:, 0:nd], start=True, stop=True)

                # copy to x3_s (+ b_lin) accumulating per-feature sums over
                # the real columns only
                blin = bl_s[:] if cfg["has_blin"] else 0.0
                nc.scalar.activation(
                    x3_s[:, d0:d0 + ndr], x3_ps[:, 0:ndr],
                    mybir.ActivationFunctionType.Identity,
                    bias=blin, scale=1.0,
                    accum_out=sums[:, ci:ci + 1])
                junk = misc.tile([128, chunk * BW], BF16, tag="junk")
                nc.scalar.activation(
                    junk[:, 0:ndr], x3_s[:, d0:d0 + ndr],
                    mybir.ActivationFunctionType.Square,
                    accum_out=sqs[:, ci:ci + 1])

            if NPC < npc_pad:
                nc.vector.memset(x3_s[:, NPC:npc_pad], 0.0)

            # ---- global BN statistics via AllReduce ----
            stat2 = consts.tile([128, 2], F32)
            nc.vector.tensor_reduce(stat2[:, 0:1], sums[:, 0:nchunks],
                                    axis=mybir.AxisListType.X,
                                    op=mybir.AluOpType.add)
            nc.vector.tensor_reduce(stat2[:, 1:2], sqs[:, 0:nchunks],
                                    axis=mybir.AxisListType.X,
                                    op=mybir.AluOpType.add)
            cc_in = dram.tile([128, 2], F32)
            cc_out = dram.tile([128, 2], F32)
            nc.gpsimd.dma_start(cc_in[:], stat2[:])
            if ncores > 1 and not cfg.get("no_cc"):
                nc.gpsimd.collective_compute(
                    "AllReduce",
                    mybir.AluOpType.add,
                    replica_groups=[list(range(ncores))],
                    ins=[cc_in[:].opt()],
                    outs=[cc_out[:].opt()],
                )
                red = cc_out
            else:
                red = cc_in
            stat_r = consts.tile([128, 2], F32)
            nc.sync.dma_start(stat_r[:], red[:])

            mean = consts.tile([128, 1], F32)
            ex2 = consts.tile([128, 1], F32)
            var = consts.tile([128, 1], F32)
            rstd = consts.tile([128, 1], F32)
            scl = consts.tile([128, 1], F32)
            bia = consts.tile([128, 1], F32)
            tmp1 = consts.tile([128, 1], F32)
            nc.vector.tensor_scalar_mul(mean[:], stat_r[:, 0:1], inv_n)
            nc.vector.tensor_scalar_mul(ex2[:], stat_r[:, 1:2], inv_n)
            nc.vector.tensor_tensor(out=tmp1[:], in0=mean[:], in1=mean[:],
                                    op=mybir.AluOpType.mult)
            nc.vector.tensor_sub(var[:], ex2[:], tmp1[:])
            epsv = consts.tile([128, 1], F32)
            nc.vector.memset(epsv[:], BN_EPS)
            nc.scalar.activation(rstd[:], var[:],
                                 mybir.ActivationFunctionType.Sqrt,
                                 bias=epsv[:], scale=1.0)
            nc.vector.reciprocal(rstd[:], rstd[:])
            nc.vector.tensor_tensor(out=scl[:], in0=ga_s[:], in1=rstd[:],
                                    op=mybir.AluOpType.mult)
            nc.vector.tensor_tensor(out=tmp1[:], in0=mean[:], in1=scl[:],
                                    op=mybir.AluOpType.mult)
            nc.vector.tensor_sub(bia[:], be_s[:], tmp1[:])

            # ---- normalize + leaky (batched), transpose, store bf16 ----
            npairs = npc_pad // 128
            QN = 4
            qpairs = (npairs + QN - 1) // QN
            done = 0
            for q in range(QN):
                pq = min(qpairs, npairs - done)
                if pq <= 0:
                    break
                lo = done * 128
                hi = (done + pq) * 128
                nc.scalar.activation(x3_s[:, lo:hi], x3_s[:, lo:hi],
                                     mybir.ActivationFunctionType.Identity,
                                     bias=bia[:], scale=scl[:])
                nc.vector.scalar_tensor_tensor(
                    out=x3_s[:, lo:hi], in0=x3_s[:, lo:hi], scalar=NEG,
                    in1=x3_s[:, lo:hi],
                    op0=mybir.AluOpType.mult, op1=mybir.AluOpType.max)
                for p in range(done, done + pq):
                    x3_blk = x3_s[:, p * 128:(p + 1) * 128]
                    tr_ps = tp.tile([128, 128], F32, tag="tr")
                    nc.tensor.transpose(tr_ps[:], x3_blk, ident[:])
                    o_sb = misc.tile([128, 128], BF16, tag="osb")
                    nc.vector.tensor_copy(o_sb[:], tr_ps[:])
                    nc.sync.dma_start(out_d[p * 128:(p + 1) * 128, :],
                                      o_sb[:])
                done += pq

    nc.compile()
    return nc


def preprocess(x, edge_index, cfg):
    """Host-side sharding: balanced dst blocks + per-core edge/index arrays.

    Returns (per_core_inputs, perm) where perm[c] maps padded slot positions
    to global node ids (for unpermuting the output on the host).
    """
    ncores = cfg["n_cores"]
    nblk = cfg["nblk"]
    n = x.shape[0]
    npc = cfg["npc"]
    npc_pad = nblk * BW

    src = np.asarray(edge_index[0], dtype=np.int64)
    dst = np.asarray(edge_index[1], dtype=np.int64)
    core = dst // npc
    loc = dst - core * npc

    xb = np.zeros((ROWS_PAD, 128), dtype=ml_dtypes.bfloat16)
    xb[:n] = x.astype(ml_dtypes.bfloat16)

    # per-(core, loc) degree
    deg = np.zeros((ncores, npc), dtype=np.int64)
    np.add.at(deg, (core, loc), 1)

    blk_of = np.empty((ncores, npc), dtype=np.int64)
    pos_of = np.empty((ncores, npc), dtype=np.int64)
    caps = np.full(nblk, BW, dtype=np.int64)
    caps[nblk - 1] = LAST_BLK
    l_need = 0
    for c in range(ncores):
        order = np.argsort(-deg[c], kind="stable")
        sums_b = np.zeros(nblk, dtype=np.int64)
        fill = np.zeros(nblk, dtype=np.int64)
        # greedy rounds: place the next batch of highest-degree nodes onto
        # the currently-lightest capacity-open blocks (near-LPT)
        ptr = 0
        while ptr < npc:
            open_b = np.where(fill < caps)[0]
            k = min(len(open_b), npc - ptr)
            ob = open_b[np.argsort(sums_b[open_b], kind="stable")[:k]]
            nodes = order[ptr:ptr + k]
            blk_of[c, nodes] = ob
            pos_of[c, nodes] = fill[ob]
            sums_b[ob] += deg[c, nodes]
            fill[ob] += 1
            ptr += k
        l_need = max(l_need, int(np.ceil(sums_b.max() / 128)))
    cfg["l_max"] = max(l_need, 1)
    L = cfg["l_max"]
    nslots = nblk * L

    # per-edge placement
    e_blk = blk_of[core, loc]
    e_dloc = pos_of[core, loc]
    key = core * nblk + e_blk
    order_e = np.argsort(key, kind="stable")
    key_s = key[order_e]
    src_s = src[order_e]
    dloc_s = e_dloc[order_e]
    ngroups = ncores * nblk
    counts = np.bincount(key_s, minlength=ngroups)
    starts = np.zeros(ngroups + 1, dtype=np.int64)
    np.cumsum(counts, out=starts[1:])
    j = np.arange(len(src_s)) - starts[key_s]   # rank within (core, blk)
    c_e = key_s // nblk
    b_e = key_s % nblk

    idx_all = np.zeros((ncores, nblk, L * 128), dtype=np.int32)
    dv_all = np.full((ncores, nblk, L * 128), 255, dtype=np.int64)
    idx_all[c_e, b_e, j] = src_s
    dv_all[c_e, b_e, j] = dloc_s

    per_core = []
    perm = np.full((ncores, npc_pad), -1, dtype=np.int64)
    for c in range(ncores):
        # idx layout: [128 lanes, nblk*L slots]; edge j in (c, b) sits at
        # slot b*L + j//128, lane j%128
        iw = idx_all[c].reshape(nblk, L, 128)          # [b, t, lane]
        dw = dv_all[c].reshape(nblk, L, 128)
        idx = np.ascontiguousarray(
            iw.transpose(2, 0, 1).reshape(128, nslots))
        dv = np.ascontiguousarray(
            dw.transpose(2, 0, 1).reshape(128, nslots))
        # own nodes, transposed, in permuted order
        nodes = np.arange(npc, dtype=np.int64)
        slot = blk_of[c] * BW + pos_of[c]
        perm[c, slot] = nodes + c * npc
        xoT = np.zeros((128, npc_pad), dtype=ml_dtypes.bfloat16)
        xoT[:, slot] = xb[nodes + c * npc].T
        per_core.append({
            "xb": xb,
            "x_ownT": xoT,
            "idx": idx,
            "dvals": dv.astype(ml_dtypes.bfloat16),
        })
    return per_core, perm


_PROGRAM_CACHE = {}


def run(x, edge_index, W_rel, b_rel, W_root, W_lin, b_lin, gamma, beta, cfg):
    per_core, perm = preprocess(x, edge_index, cfg)
    cfg["has_brel"] = bool(np.any(b_rel != 0))
    cfg["has_blin"] = bool(np.any(b_lin != 0))

    iota = np.tile(np.arange(128, dtype=np.float32), (128, 1))
    shared = {
        "iota": iota.astype(ml_dtypes.bfloat16),
        "WrT": np.ascontiguousarray(W_rel.T).astype(ml_dtypes.bfloat16),
        "WoT": np.ascontiguousarray(W_root.T).astype(ml_dtypes.bfloat16),
        "WlT": np.ascontiguousarray(W_lin.T).astype(ml_dtypes.bfloat16),
        "brel": b_rel.reshape(128, 1).astype(np.float32),
        "blin": b_lin.reshape(128, 1).astype(np.float32),
        "gamma": gamma.reshape(128, 1).astype(np.float32),
        "beta": beta.reshape(128, 1).astype(np.float32),
    }
    in_maps = [dict(m, **shared) for m in per_core]

    key = (cfg["n_cores"], cfg["nblk"], cfg["l_max"], cfg["chunk"],
           cfg["has_brel"], cfg["has_blin"])
    if key not in _PROGRAM_CACHE:
        _PROGRAM_CACHE[key] = build_program(cfg)
    nc = _PROGRAM_CACHE[key]

    res = bass_utils.run_bass_kernel_spmd(
        nc, in_maps, core_ids=list(range(cfg["n_cores"])))
    n = x.shape[0]
    out = np.empty((n, 128), dtype=np.float32)
    for c in range(cfg["n_cores"]):
        o = np.asarray(res.results[c]["out"]).astype(np.float32)
        m = perm[c] >= 0
        out[perm[c][m]] = o[m]
    return out


def kernel(x, edge_index, batch, W_rel, b_rel, W_root, W_lin, b_lin, gamma,
           beta):
    x = np.asarray(x, dtype=np.float32)
    cfg = {
        "n_cores": N_CORES,
        "npc": NPC,
        "nblk": NBLK,
        "chunk": CHUNK,
        "n_total": N_NODES,
    }
    return run(x, np.asarray(edge_index), np.asarray(W_rel, dtype=np.float32),
               np.asarray(b_rel, dtype=np.float32),
               np.asarray(W_root, dtype=np.float32),
               np.asarray(W_lin, dtype=np.float32),
               np.asarray(b_lin, dtype=np.float32),
               np.asarray(gamma, dtype=np.float32),
               np.asarray(beta, dtype=np.float32), cfg)
